# revision 1
# baseline (speedup 1.0000x reference)
"""Distributed Trainium2 kernel for a single causal attention head (v3).

v6 changes vs v5 (~271-273 us, PE steady-state ~97% but 18.9 us head
+ 11.5 us tail):
  - constants sent pre-arranged from host (plain 2D DMAs, no strided
    rearrange descriptors).
  - first-pair x loads issued before the const DMAs, split in halves so
    the first projection matmuls start ~1.5 us earlier.
  - PE warmup matmuls on a memset scratch tile during the initial DMA
    wait (HAM un-throttles before real work arrives).
  - out-store back on the Sync HWDGE ring.
  - x loads prefetched 2 pipeline iterations ahead (xpool bufs=6).
  - ew for both batches in one tile; causal mask applied in a single
    broadcast DVE multiply.

v3 changes vs v2 (v2 measured 310 us, PE-bound 90%, ACT 84%):
  - V projection col-packed across block PAIRS: block 2m's v at output
    partitions 0-63 (col groups 0-1), block 2m+1's at 64-127 (2-3),
    concurrently in the PE array. Halves V-projection PE time.
  - Fewer, bigger DVE/ACT instructions (per-instr overhead ~150-460ns):
    one reciprocal per block over [128,4,1], one merged vt copy per
    transpose, one mask-mul per batch via [128,3,128] mask (ones in the
    middle k0xq1 tile).
  - Softmax normalization moved from ScalarE to VectorE via stride-0
    broadcast multiply.
  - Out DMA moved to the idle Sync HWDGE ring.
"""

import os
import sys

sys.path.insert(0, os.environ.get("TRN_RL_REPO", "/opt/trn_rl_repo"))

from contextlib import ExitStack

import numpy as np
import ml_dtypes

import concourse.bass as bass
import concourse.tile as tile
from concourse import bacc, mybir

F32 = mybir.dt.float32
BF16 = mybir.dt.bfloat16
AF = mybir.ActivationFunctionType

N_CORES = 8
B, T, C, H = 1024, 256, 1024, 64
B_SH = B // N_CORES          # 128 batches per core
TOK = B_SH * T               # 32768 tokens per core
TB = 512                     # tokens per block (2 batches)
NBLK = TOK // TB             # 64 (must be even: paired blocks)
NC_TILES = C // 128          # 8 contraction tiles
SCALE = 1.0 / float(np.sqrt(np.float32(C)))


def build_graph(nblk=NBLK):
    assert nblk % 2 == 0
    nc = bacc.Bacc("TRN2", target_bir_lowering=False, debug=False)

    xb = nc.dram_tensor("xb", [128, nblk * NC_TILES * TB], BF16,
                        kind="ExternalInput").ap()
    wqk_d = nc.dram_tensor("wqk", [128, NC_TILES * 128], BF16,
                           kind="ExternalInput").ap()
    wv_d = nc.dram_tensor("wv", [128, NC_TILES * H], BF16,
                          kind="ExternalInput").ap()
    id_d = nc.dram_tensor("ident", [128, 128], BF16, kind="ExternalInput").ap()
    mask_d = nc.dram_tensor("mask3", [128, 3 * 128], BF16,
                            kind="ExternalInput").ap()
    out_d = nc.dram_tensor("out", [128, nblk * 4 * H], BF16,
                           kind="ExternalOutput").ap()

    xb4 = xb.rearrange("p (n a t) -> p n a t", a=NC_TILES, t=TB)

    with tile.TileContext(nc) as tc, ExitStack() as ctx:
        const = ctx.enter_context(tc.tile_pool(name="const", bufs=1))
        xpool = ctx.enter_context(tc.tile_pool(name="x", bufs=6))
        ps_qk = ctx.enter_context(tc.tile_pool(name="ps_qk", bufs=2, space="PSUM"))
        ps_v = ctx.enter_context(tc.tile_pool(name="ps_v", bufs=1, space="PSUM"))
        ps_wei = ctx.enter_context(tc.tile_pool(name="ps_wei", bufs=2, space="PSUM"))
        ps_t = ctx.enter_context(tc.tile_pool(name="ps_t", bufs=1, space="PSUM"))
        ps_o = ctx.enter_context(tc.tile_pool(name="ps_o", bufs=2, space="PSUM"))
        sb_qs = ctx.enter_context(tc.tile_pool(name="sb_qs", bufs=4))
        sb_ks = ctx.enter_context(tc.tile_pool(name="sb_ks", bufs=4))
        sb_vs = ctx.enter_context(tc.tile_pool(name="sb_vs", bufs=4))
        sb_ew = ctx.enter_context(tc.tile_pool(name="sb_ew", bufs=4))
        sb_r = ctx.enter_context(tc.tile_pool(name="sb_r", bufs=4))
        sb_of = ctx.enter_context(tc.tile_pool(name="sb_of", bufs=4))

        # ---- first block's x + the qk weights first (proj needs both),
        # then block 1, then the rest of the constants ----
        xt0 = xpool.tile([128, NC_TILES, TB], BF16, tag="xt")
        nc.sync.dma_start(xt0[:, 0:4], xb4[:, 0, 0:4])
        nc.sync.dma_start(xt0[:, 4:8], xb4[:, 0, 4:8])
        wqk_t = const.tile([128, NC_TILES, 128], BF16)
        nc.sync.dma_start(wqk_t[:], wqk_d.rearrange("p (a m) -> p a m", m=128))
        xt1 = xpool.tile([128, NC_TILES, TB], BF16, tag="xt")
        nc.sync.dma_start(xt1[:, 0:4], xb4[:, 1, 0:4])
        nc.sync.dma_start(xt1[:, 4:8], xb4[:, 1, 4:8])
        wv_t = const.tile([128, NC_TILES, H], BF16)
        nc.sync.dma_start(wv_t[:], wv_d.rearrange("p (a m) -> p a m", m=H))
        id_t = const.tile([128, 128], BF16)
        nc.sync.dma_start(id_t[:], id_d[:])
        mask_t = const.tile([128, 3, 128], BF16)
        nc.sync.dma_start(mask_t[:], mask_d.rearrange("p (a b) -> p a b", b=128))

        # ---- PE warmup on a memset scratch during the DMA wait ----
        warm = const.tile([128, 512], BF16)
        nc.gpsimd.memset(warm[:], 0.0)
        warm_ps = ps_qk.tile([128, 512], F32, tag="qk", name="warm_ps")
        for w in range(20):
            nc.tensor.matmul(warm_ps[:], warm[:, 0:128], warm[:],
                             start=True, stop=True)
        # v-transpose tiles [128, 2, 65]: [:, b, 0:64]=v_nat batch b,
        # [:, b, 64]=ones (set once; copies never touch it)
        vt_tiles = [const.tile([128, 2, 65], BF16, name=f"vt{i}")
                    for i in range(8)]
        for vt in vt_tiles:
            nc.gpsimd.memset(vt[:, 0, 64:65], 1.0)
            nc.gpsimd.memset(vt[:, 1, 64:65], 1.0)

        # ---------- pipeline stages ----------
        def stage_load(blk):
            xt = xpool.tile([128, NC_TILES, TB], BF16, tag="xt")
            nc.sync.dma_start(xt[:], xb4[:, blk])
            return xt

        def stage_proj_pair(xts):
            """QKV projection for a block pair + restacking copies."""
            qk_pss = []
            for xt in xts:
                qk_ps = ps_qk.tile([128, TB], F32, tag="qk")
                for c in range(NC_TILES):
                    nc.tensor.matmul(qk_ps[:], wqk_t[:, c, :], xt[:, c, :],
                                     start=(c == 0), stop=(c == NC_TILES - 1))
                qk_pss.append(qk_ps)
            # v for both blocks, col-packed (block0 -> partitions 0:64,
            # block1 -> 64:128), concurrent in the PE array
            v_ps = ps_v.tile([128, TB], F32)
            for c in range(NC_TILES):
                nc.tensor.matmul(v_ps[0:64, :], wv_t[:, c, :], xts[0][:, c, :],
                                 start=(c == 0), stop=(c == NC_TILES - 1),
                                 skip_group_check=True)
                nc.tensor.matmul(v_ps[64:128, :], wv_t[:, c, :], xts[1][:, c, :],
                                 start=(c == 0), stop=(c == NC_TILES - 1),
                                 skip_group_check=True)
            out = []
            for half, qk_ps in enumerate(qk_pss):
                p0, p1 = 64 * half, 64 * half + 64
                qs = sb_qs.tile([128, T], BF16, name=f"qs{half}")
                nc.scalar.copy(qs[0:64, :], qk_ps[0:64, 0:T])
                nc.vector.tensor_copy(qs[64:128, :], qk_ps[0:64, T:2 * T])
                ks = sb_ks.tile([128, T], BF16, name=f"ks{half}")
                nc.vector.tensor_copy(ks[0:64, :], qk_ps[64:128, 0:T])
                nc.scalar.copy(ks[64:128, :], qk_ps[64:128, T:2 * T])
                vs = sb_vs.tile([128, T], BF16, name=f"vs{half}")
                nc.scalar.copy(vs[0:64, :], v_ps[p0:p1, 0:T])
                nc.vector.tensor_copy(vs[64:128, :], v_ps[p0:p1, T:2 * T])
                out.append((qs, ks, vs))
            return out

        def stage_wei(blk, qs, ks, vs):
            """wei matmuls (row-packed over batches), exp+mask, v transposes."""
            wei_ps = [ps_wei.tile([128, 384], F32, tag="wei", name=f"wei{b}")
                      for b in range(2)]
            vts = [vt_tiles[(blk * 2 + j) % 8] for j in range(2)]

            tp0 = ps_t.tile([128, 128], BF16, tag="tp")
            nc.tensor.transpose(tp0[:], vs[:, 0:128], id_t[:])
            nc.vector.tensor_copy(vts[0][:, :, 0:64],
                                  tp0.rearrange("p (j c) -> p j c", c=64))
            for b in range(2):           # k-tile 0, row-group packed
                p0 = 64 * b
                nc.tensor.matmul(wei_ps[b][:, 0:256], ks[p0:p0 + 64, 0:128],
                                 qs[p0:p0 + 64, 0:256], start=True, stop=True)
            tp1 = ps_t.tile([128, 128], BF16, tag="tp")
            nc.tensor.transpose(tp1[:], vs[:, 128:256], id_t[:])
            nc.vector.tensor_copy(vts[1][:, :, 0:64],
                                  tp1.rearrange("p (j c) -> p j c", c=64))
            for b in range(2):           # k-tile 1 x q-tile 1, packed
                p0 = 64 * b
                nc.tensor.matmul(wei_ps[b][:, 256:384], ks[p0:p0 + 64, 128:256],
                                 qs[p0:p0 + 64, 128:256], start=True, stop=True)
            ew2 = sb_ew.tile([128, 2, 3, 128], BF16)
            for b in range(2):
                nc.scalar.activation(ew2[:, b], wei_ps[b][:], AF.Exp,
                                     scale=SCALE)
            mask_b = mask_t[:].unsqueeze(1).broadcast_to([128, 2, 3, 128])
            nc.vector.tensor_mul(ew2[:], ew2[:], mask_b)
            return ew2, vts

        def stage_out(blk, ew2, vts):
            """o matmuls (natural orientation), normalize, store."""
            o4 = ps_o.tile([128, 4, 65], F32, tag="o")
            for b in range(2):
                ew = ew2[:, b]
                nc.tensor.matmul(o4[:, 2 * b, :], ew[:, 0, :], vts[0][:, b, :],
                                 start=True, stop=True)
                nc.tensor.matmul(o4[:, 2 * b + 1, :], ew[:, 1, :],
                                 vts[0][:, b, :], start=True, stop=False)
                nc.tensor.matmul(o4[:, 2 * b + 1, :], ew[:, 2, :],
                                 vts[1][:, b, :], start=False, stop=True)
            r = sb_r.tile([128, 4, 1], F32)
            nc.vector.reciprocal(r[:], o4[:, :, 64:65])
            of = sb_of.tile([128, 4, H], BF16)
            nc.vector.tensor_mul(of[:], o4[:, :, 0:64],
                                 r.broadcast_to([128, 4, H]))
            nc.sync.dma_start(out_d[:, blk * 4 * H:(blk + 1) * 4 * H], of[:])

        # ---------- software-pipelined main loop (pair granularity) ----
        npair = nblk // 2
        live = {0: {"xts": [xt0, xt1]}}
        for i in range(npair + 4):
            if 1 <= i < npair:
                live[i] = {"xts": [stage_load(2 * i), stage_load(2 * i + 1)]}
            if 2 <= i <= npair + 1:
                m = i - 2
                live[m]["pqv"] = stage_proj_pair(live[m].pop("xts"))
            if 3 <= i <= npair + 2:
                m = i - 3
                pqv = live[m].pop("pqv")
                live[m]["wt"] = [stage_wei(2 * m + h, *pqv[h]) for h in (0, 1)]
            if 4 <= i <= npair + 3:
                m = i - 4
                for h in (0, 1):
                    ew2, vts = live[m]["wt"][h]
                    stage_out(2 * m + h, ew2, vts)
                del live[m]

    nc.compile()
    return nc


_GRAPH = None


def _get_graph():
    global _GRAPH
    if _GRAPH is None:
        _GRAPH = build_graph()
    return _GRAPH


def _make_consts():
    ident = np.eye(128, dtype=np.float32).astype(ml_dtypes.bfloat16)
    # mask3[p, 0, q] = within-tile causal (q >= p); [:, 1, :] = ones
    # (k0 x q1 full tile); [:, 2, :] = causal again (k1 x q1 diag)
    tri = np.triu(np.ones((128, 128), dtype=np.float32))
    mask3 = np.stack([tri, np.ones((128, 128), dtype=np.float32), tri], axis=1)
    mask3 = mask3.reshape(128, 3 * 128).astype(ml_dtypes.bfloat16)
    return ident, mask3


def _prep_x_shard(xsh):
    """(B_SH, T, C) f32 -> [128, NBLK*8*TB] bf16 block-contiguous layout."""
    xs = np.asarray(xsh, dtype=np.float32).reshape(TOK, C)
    xs = xs.astype(ml_dtypes.bfloat16)
    xs = xs.reshape(NBLK, TB, NC_TILES, 128).transpose(3, 0, 2, 1)
    return np.ascontiguousarray(xs).reshape(128, NBLK * NC_TILES * TB)


def make_in_maps(x, Wq, Wk, Wv):
    ident, mask3 = _make_consts()
    wqk = np.concatenate([np.asarray(Wq), np.asarray(Wk)], axis=1)
    wqk = wqk.astype(ml_dtypes.bfloat16)          # [C, 128]
    # -> [128, 8, 128]: [p, a, m] = wqk[a*128+p, m]
    wqk = np.ascontiguousarray(
        wqk.reshape(NC_TILES, 128, 128).transpose(1, 0, 2)).reshape(128, -1)
    wv = np.asarray(Wv).astype(ml_dtypes.bfloat16)
    wv = np.ascontiguousarray(
        wv.reshape(NC_TILES, 128, H).transpose(1, 0, 2)).reshape(128, -1)

    x = np.asarray(x, dtype=np.float32)
    in_maps = []
    for i in range(N_CORES):
        in_maps.append({
            "xb": _prep_x_shard(x[i * B_SH:(i + 1) * B_SH]),
            "wqk": wqk, "wv": wv, "ident": ident, "mask3": mask3,
        })
    return in_maps


def _unpack_out(o):
    """[128, NBLK*4*H] bf16 -> (B_SH, T, H) f32.  token = blk*512+j*128+p."""
    o = np.asarray(o).reshape(128, NBLK, 4, H).transpose(1, 2, 0, 3)
    return np.ascontiguousarray(o).reshape(B_SH, T, H).astype(np.float32)


def _run(x, Wq, Wk, Wv, trace=False):
    from concourse.bass_utils import run_bass_kernel_spmd

    nc = _get_graph()
    in_maps = make_in_maps(x, Wq, Wk, Wv)
    res = run_bass_kernel_spmd(nc, in_maps, list(range(N_CORES)), trace=trace)
    full = np.empty((B, T, H), dtype=np.float32)
    for i in range(N_CORES):
        full[i * B_SH:(i + 1) * B_SH] = _unpack_out(res.results[i]["out"])
    return full, res


def kernel(x, Wq, Wk, Wv):
    full, _ = _run(x, Wq, Wk, Wv, trace=False)
    return full


if __name__ == "__main__":
    build_graph()
    print("graph built + compiled OK")



# revision 2
# speedup vs baseline: 1.0055x; 1.0055x over previous
"""Distributed Trainium2 kernel for a single causal attention head (v3).

v6b changes vs v6 (measured 265.6/266.0 us):
  - warmup matmuls issued before any DMA dependency (DVE memset scratch)
    so the PE spins up at engine start.
  - x prefetch deepened to 8 block buffers.

v6 changes vs v5 (~271-273 us, PE steady-state ~97% but 18.9 us head
+ 11.5 us tail):
  - constants sent pre-arranged from host (plain 2D DMAs, no strided
    rearrange descriptors).
  - first-pair x loads issued before the const DMAs, split in halves so
    the first projection matmuls start ~1.5 us earlier.
  - PE warmup matmuls on a memset scratch tile during the initial DMA
    wait (HAM un-throttles before real work arrives).
  - out-store back on the Sync HWDGE ring.
  - x loads prefetched 2 pipeline iterations ahead (xpool bufs=6).
  - ew for both batches in one tile; causal mask applied in a single
    broadcast DVE multiply.

v3 changes vs v2 (v2 measured 310 us, PE-bound 90%, ACT 84%):
  - V projection col-packed across block PAIRS: block 2m's v at output
    partitions 0-63 (col groups 0-1), block 2m+1's at 64-127 (2-3),
    concurrently in the PE array. Halves V-projection PE time.
  - Fewer, bigger DVE/ACT instructions (per-instr overhead ~150-460ns):
    one reciprocal per block over [128,4,1], one merged vt copy per
    transpose, one mask-mul per batch via [128,3,128] mask (ones in the
    middle k0xq1 tile).
  - Softmax normalization moved from ScalarE to VectorE via stride-0
    broadcast multiply.
  - Out DMA moved to the idle Sync HWDGE ring.
"""

import os
import sys

sys.path.insert(0, os.environ.get("TRN_RL_REPO", "/opt/trn_rl_repo"))

from contextlib import ExitStack

import numpy as np
import ml_dtypes

import concourse.bass as bass
import concourse.tile as tile
from concourse import bacc, mybir

F32 = mybir.dt.float32
BF16 = mybir.dt.bfloat16
AF = mybir.ActivationFunctionType

N_CORES = 8
B, T, C, H = 1024, 256, 1024, 64
B_SH = B // N_CORES          # 128 batches per core
TOK = B_SH * T               # 32768 tokens per core
TB = 512                     # tokens per block (2 batches)
NBLK = TOK // TB             # 64 (must be even: paired blocks)
NC_TILES = C // 128          # 8 contraction tiles
SCALE = 1.0 / float(np.sqrt(np.float32(C)))


def build_graph(nblk=NBLK):
    assert nblk % 2 == 0
    nc = bacc.Bacc("TRN2", target_bir_lowering=False, debug=False)

    xb = nc.dram_tensor("xb", [128, nblk * NC_TILES * TB], BF16,
                        kind="ExternalInput").ap()
    wqk_d = nc.dram_tensor("wqk", [128, NC_TILES * 128], BF16,
                           kind="ExternalInput").ap()
    wv_d = nc.dram_tensor("wv", [128, NC_TILES * H], BF16,
                          kind="ExternalInput").ap()
    id_d = nc.dram_tensor("ident", [128, 128], BF16, kind="ExternalInput").ap()
    mask_d = nc.dram_tensor("mask3", [128, 3 * 128], BF16,
                            kind="ExternalInput").ap()
    out_d = nc.dram_tensor("out", [128, nblk * 4 * H], BF16,
                           kind="ExternalOutput").ap()

    xb4 = xb.rearrange("p (n a t) -> p n a t", a=NC_TILES, t=TB)

    with tile.TileContext(nc) as tc, ExitStack() as ctx:
        const = ctx.enter_context(tc.tile_pool(name="const", bufs=1))
        xpool = ctx.enter_context(tc.tile_pool(name="x", bufs=8))
        ps_qk = ctx.enter_context(tc.tile_pool(name="ps_qk", bufs=2, space="PSUM"))
        ps_v = ctx.enter_context(tc.tile_pool(name="ps_v", bufs=1, space="PSUM"))
        ps_wei = ctx.enter_context(tc.tile_pool(name="ps_wei", bufs=2, space="PSUM"))
        ps_t = ctx.enter_context(tc.tile_pool(name="ps_t", bufs=1, space="PSUM"))
        ps_o = ctx.enter_context(tc.tile_pool(name="ps_o", bufs=2, space="PSUM"))
        sb_qs = ctx.enter_context(tc.tile_pool(name="sb_qs", bufs=4))
        sb_ks = ctx.enter_context(tc.tile_pool(name="sb_ks", bufs=4))
        sb_vs = ctx.enter_context(tc.tile_pool(name="sb_vs", bufs=4))
        sb_ew = ctx.enter_context(tc.tile_pool(name="sb_ew", bufs=4))
        sb_r = ctx.enter_context(tc.tile_pool(name="sb_r", bufs=4))
        sb_of = ctx.enter_context(tc.tile_pool(name="sb_of", bufs=4))

        # ---- PE warmup immediately: matmuls on an uninitialized scratch
        # tile (outputs never read) so HAM un-throttles before real work;
        # no memset dependency so these issue at t~0 ----
        warm = const.tile([128, 512], BF16)
        nc.vector.memset(warm[:], 0.0)
        warm_ps = ps_qk.tile([128, 512], F32, tag="qk", name="warm_ps")
        for w in range(20):
            nc.tensor.matmul(warm_ps[:], warm[:, 0:128], warm[:],
                             start=True, stop=True)

        # ---- first block's x + the qk weights first (proj needs both),
        # then block 1, then the rest of the constants ----
        xt0 = xpool.tile([128, NC_TILES, TB], BF16, tag="xt")
        nc.sync.dma_start(xt0[:, 0:4], xb4[:, 0, 0:4])
        nc.sync.dma_start(xt0[:, 4:8], xb4[:, 0, 4:8])
        wqk_t = const.tile([128, NC_TILES, 128], BF16)
        nc.sync.dma_start(wqk_t[:], wqk_d.rearrange("p (a m) -> p a m", m=128))
        xt1 = xpool.tile([128, NC_TILES, TB], BF16, tag="xt")
        nc.sync.dma_start(xt1[:, 0:4], xb4[:, 1, 0:4])
        nc.sync.dma_start(xt1[:, 4:8], xb4[:, 1, 4:8])
        wv_t = const.tile([128, NC_TILES, H], BF16)
        nc.sync.dma_start(wv_t[:], wv_d.rearrange("p (a m) -> p a m", m=H))
        id_t = const.tile([128, 128], BF16)
        nc.sync.dma_start(id_t[:], id_d[:])
        mask_t = const.tile([128, 3, 128], BF16)
        nc.sync.dma_start(mask_t[:], mask_d.rearrange("p (a b) -> p a b", b=128))

        # v-transpose tiles [128, 2, 65]: [:, b, 0:64]=v_nat batch b,
        # [:, b, 64]=ones (set once; copies never touch it)
        vt_tiles = [const.tile([128, 2, 65], BF16, name=f"vt{i}")
                    for i in range(8)]
        for vt in vt_tiles:
            nc.gpsimd.memset(vt[:, 0, 64:65], 1.0)
            nc.gpsimd.memset(vt[:, 1, 64:65], 1.0)

        # ---------- pipeline stages ----------
        def stage_load(blk):
            xt = xpool.tile([128, NC_TILES, TB], BF16, tag="xt")
            nc.sync.dma_start(xt[:], xb4[:, blk])
            return xt

        def stage_proj_pair(xts):
            """QKV projection for a block pair + restacking copies."""
            qk_pss = []
            for xt in xts:
                qk_ps = ps_qk.tile([128, TB], F32, tag="qk")
                for c in range(NC_TILES):
                    nc.tensor.matmul(qk_ps[:], wqk_t[:, c, :], xt[:, c, :],
                                     start=(c == 0), stop=(c == NC_TILES - 1))
                qk_pss.append(qk_ps)
            # v for both blocks, col-packed (block0 -> partitions 0:64,
            # block1 -> 64:128), concurrent in the PE array
            v_ps = ps_v.tile([128, TB], F32)
            for c in range(NC_TILES):
                nc.tensor.matmul(v_ps[0:64, :], wv_t[:, c, :], xts[0][:, c, :],
                                 start=(c == 0), stop=(c == NC_TILES - 1),
                                 skip_group_check=True)
                nc.tensor.matmul(v_ps[64:128, :], wv_t[:, c, :], xts[1][:, c, :],
                                 start=(c == 0), stop=(c == NC_TILES - 1),
                                 skip_group_check=True)
            out = []
            for half, qk_ps in enumerate(qk_pss):
                p0, p1 = 64 * half, 64 * half + 64
                qs = sb_qs.tile([128, T], BF16, name=f"qs{half}")
                nc.scalar.copy(qs[0:64, :], qk_ps[0:64, 0:T])
                nc.vector.tensor_copy(qs[64:128, :], qk_ps[0:64, T:2 * T])
                ks = sb_ks.tile([128, T], BF16, name=f"ks{half}")
                nc.vector.tensor_copy(ks[0:64, :], qk_ps[64:128, 0:T])
                nc.scalar.copy(ks[64:128, :], qk_ps[64:128, T:2 * T])
                vs = sb_vs.tile([128, T], BF16, name=f"vs{half}")
                nc.scalar.copy(vs[0:64, :], v_ps[p0:p1, 0:T])
                nc.vector.tensor_copy(vs[64:128, :], v_ps[p0:p1, T:2 * T])
                out.append((qs, ks, vs))
            return out

        def stage_wei(blk, qs, ks, vs):
            """wei matmuls (row-packed over batches), exp+mask, v transposes."""
            wei_ps = [ps_wei.tile([128, 384], F32, tag="wei", name=f"wei{b}")
                      for b in range(2)]
            vts = [vt_tiles[(blk * 2 + j) % 8] for j in range(2)]

            tp0 = ps_t.tile([128, 128], BF16, tag="tp")
            nc.tensor.transpose(tp0[:], vs[:, 0:128], id_t[:])
            nc.vector.tensor_copy(vts[0][:, :, 0:64],
                                  tp0.rearrange("p (j c) -> p j c", c=64))
            for b in range(2):           # k-tile 0, row-group packed
                p0 = 64 * b
                nc.tensor.matmul(wei_ps[b][:, 0:256], ks[p0:p0 + 64, 0:128],
                                 qs[p0:p0 + 64, 0:256], start=True, stop=True)
            tp1 = ps_t.tile([128, 128], BF16, tag="tp")
            nc.tensor.transpose(tp1[:], vs[:, 128:256], id_t[:])
            nc.vector.tensor_copy(vts[1][:, :, 0:64],
                                  tp1.rearrange("p (j c) -> p j c", c=64))
            for b in range(2):           # k-tile 1 x q-tile 1, packed
                p0 = 64 * b
                nc.tensor.matmul(wei_ps[b][:, 256:384], ks[p0:p0 + 64, 128:256],
                                 qs[p0:p0 + 64, 128:256], start=True, stop=True)
            ew2 = sb_ew.tile([128, 2, 3, 128], BF16)
            for b in range(2):
                nc.scalar.activation(ew2[:, b], wei_ps[b][:], AF.Exp,
                                     scale=SCALE)
            mask_b = mask_t[:].unsqueeze(1).broadcast_to([128, 2, 3, 128])
            nc.vector.tensor_mul(ew2[:], ew2[:], mask_b)
            return ew2, vts

        def stage_out(blk, ew2, vts):
            """o matmuls (natural orientation), normalize, store."""
            o4 = ps_o.tile([128, 4, 65], F32, tag="o")
            for b in range(2):
                ew = ew2[:, b]
                nc.tensor.matmul(o4[:, 2 * b, :], ew[:, 0, :], vts[0][:, b, :],
                                 start=True, stop=True)
                nc.tensor.matmul(o4[:, 2 * b + 1, :], ew[:, 1, :],
                                 vts[0][:, b, :], start=True, stop=False)
                nc.tensor.matmul(o4[:, 2 * b + 1, :], ew[:, 2, :],
                                 vts[1][:, b, :], start=False, stop=True)
            r = sb_r.tile([128, 4, 1], F32)
            nc.vector.reciprocal(r[:], o4[:, :, 64:65])
            of = sb_of.tile([128, 4, H], BF16)
            nc.vector.tensor_mul(of[:], o4[:, :, 0:64],
                                 r.broadcast_to([128, 4, H]))
            nc.sync.dma_start(out_d[:, blk * 4 * H:(blk + 1) * 4 * H], of[:])

        # ---------- software-pipelined main loop (pair granularity) ----
        npair = nblk // 2
        live = {0: {"xts": [xt0, xt1]}}
        for i in range(npair + 4):
            if 1 <= i < npair:
                live[i] = {"xts": [stage_load(2 * i), stage_load(2 * i + 1)]}
            if 2 <= i <= npair + 1:
                m = i - 2
                live[m]["pqv"] = stage_proj_pair(live[m].pop("xts"))
            if 3 <= i <= npair + 2:
                m = i - 3
                pqv = live[m].pop("pqv")
                live[m]["wt"] = [stage_wei(2 * m + h, *pqv[h]) for h in (0, 1)]
            if 4 <= i <= npair + 3:
                m = i - 4
                for h in (0, 1):
                    ew2, vts = live[m]["wt"][h]
                    stage_out(2 * m + h, ew2, vts)
                del live[m]

    nc.compile()
    return nc


_GRAPH = None


def _get_graph():
    global _GRAPH
    if _GRAPH is None:
        _GRAPH = build_graph()
    return _GRAPH


def _make_consts():
    ident = np.eye(128, dtype=np.float32).astype(ml_dtypes.bfloat16)
    # mask3[p, 0, q] = within-tile causal (q >= p); [:, 1, :] = ones
    # (k0 x q1 full tile); [:, 2, :] = causal again (k1 x q1 diag)
    tri = np.triu(np.ones((128, 128), dtype=np.float32))
    mask3 = np.stack([tri, np.ones((128, 128), dtype=np.float32), tri], axis=1)
    mask3 = mask3.reshape(128, 3 * 128).astype(ml_dtypes.bfloat16)
    return ident, mask3


def _prep_x_shard(xsh):
    """(B_SH, T, C) f32 -> [128, NBLK*8*TB] bf16 block-contiguous layout."""
    xs = np.asarray(xsh, dtype=np.float32).reshape(TOK, C)
    xs = xs.astype(ml_dtypes.bfloat16)
    xs = xs.reshape(NBLK, TB, NC_TILES, 128).transpose(3, 0, 2, 1)
    return np.ascontiguousarray(xs).reshape(128, NBLK * NC_TILES * TB)


def make_in_maps(x, Wq, Wk, Wv):
    ident, mask3 = _make_consts()
    wqk = np.concatenate([np.asarray(Wq), np.asarray(Wk)], axis=1)
    wqk = wqk.astype(ml_dtypes.bfloat16)          # [C, 128]
    # -> [128, 8, 128]: [p, a, m] = wqk[a*128+p, m]
    wqk = np.ascontiguousarray(
        wqk.reshape(NC_TILES, 128, 128).transpose(1, 0, 2)).reshape(128, -1)
    wv = np.asarray(Wv).astype(ml_dtypes.bfloat16)
    wv = np.ascontiguousarray(
        wv.reshape(NC_TILES, 128, H).transpose(1, 0, 2)).reshape(128, -1)

    x = np.asarray(x, dtype=np.float32)
    in_maps = []
    for i in range(N_CORES):
        in_maps.append({
            "xb": _prep_x_shard(x[i * B_SH:(i + 1) * B_SH]),
            "wqk": wqk, "wv": wv, "ident": ident, "mask3": mask3,
        })
    return in_maps


def _unpack_out(o):
    """[128, NBLK*4*H] bf16 -> (B_SH, T, H) f32.  token = blk*512+j*128+p."""
    o = np.asarray(o).reshape(128, NBLK, 4, H).transpose(1, 2, 0, 3)
    return np.ascontiguousarray(o).reshape(B_SH, T, H).astype(np.float32)


def _run(x, Wq, Wk, Wv, trace=False):
    from concourse.bass_utils import run_bass_kernel_spmd

    nc = _get_graph()
    in_maps = make_in_maps(x, Wq, Wk, Wv)
    res = run_bass_kernel_spmd(nc, in_maps, list(range(N_CORES)), trace=trace)
    full = np.empty((B, T, H), dtype=np.float32)
    for i in range(N_CORES):
        full[i * B_SH:(i + 1) * B_SH] = _unpack_out(res.results[i]["out"])
    return full, res


def kernel(x, Wq, Wk, Wv):
    full, _ = _run(x, Wq, Wk, Wv, trace=False)
    return full


if __name__ == "__main__":
    build_graph()
    print("graph built + compiled OK")

